# revision 28
# baseline (speedup 1.0000x reference)
"""Trainium2 Bass kernel for the biaffine scorer (nn_Biaffine_overlap).

Reference computation (B=8, P=64, S=512, D=512, L=64):
    pred = gather(span_repr, predicates)                      # [B,P,D]
    first[b,p,s,l]  = sum_{d,e} pred[b,p,d] W1[l,d,e] span[b,s,e]
    combine         = first + rowterm[b,p,l] + colterm[b,s,l]
    out             = combine.reshape(-1, L); out[:, -1] = 0
where
    rowterm[b,p,l] = pred@W2[:D] + b + relu(pred@Wp.T+bp)@wp
    colterm[b,s,l] = span@W2[D:] + relu(span@Wa.T+ba)@wa

Sharding: tensor-parallel over the label dim L across 8 cores (8 labels
per core).  The W1 shard (8.4 MB) stays resident in SBUF; every core
processes all batches.  Per core, two dense matmul stages:
  stage 1: tmpT[e,(l,b,p)] = sum_d W1[l,d,e] predT[d,(b,p)]   (128 MMs)
  stage 2: out[(l,p),s]    = sum_e tmpT[e,(l,p)] spanT[e,s]   (128 MMs)
           + one K=9 "extension" matmul per output tile that folds in
           rowterm/colterm as a rank-9 update.
The small FFNN score terms (<6% of FLOPs) are computed on the host and
enter the device only through the rank-9 extension operands.
"""

import numpy as np

B, P, S, D, L = 8, 64, 512, 512, 64
NCORES = 8
LSH = L // NCORES      # labels per core
KEXT = LSH + 1         # extension contraction rows (8 colterm + 1 rowterm)
DC = D // 128          # contraction chunks of 128
ET = D // 128          # e-tiles of 128
NT = (LSH * P) // 128  # output row tiles per batch (4)

_CACHE: dict = {}


def _build_module(s1_dtype_name: str | None = None, s2_dtype_name: str | None = None):
    s1_dtype_name = s1_dtype_name or S1_DTYPE
    s2_dtype_name = s2_dtype_name or S2_DTYPE
    import concourse.bacc as bacc
    import concourse.mybir as mybir
    import concourse.tile as tile

    dt = mybir.dt
    f32 = dt.float32
    s1dt = getattr(dt, s1_dtype_name)   # stage-1 operand dtype (W1, predT)
    s2dt = getattr(dt, s2_dtype_name)   # stage-2 operand dtype (tmpT, spanT, ext)

    nc = bacc.Bacc(None, target_bir_lowering=False)

    # inputs are host-packed so every DMA lands in SBUF layout with long
    # contiguous per-partition runs: [128, DC, width] with partition p
    # holding row d = c*128 + p of the logical [D, width] matrix
    spanT_d = nc.declare_dram_parameter("spanT", [B, 128, DC * S], s2dt, isOutput=False)
    predT_d = nc.declare_dram_parameter("predT", [128, DC * B * P], s1dt, isOutput=False)
    w1s_d = nc.declare_dram_parameter("w1s", [LSH, 128, DC * D], s1dt, isOutput=False)
    extL_d = nc.declare_dram_parameter("extL", [KEXT, B, NT, 128], s2dt, isOutput=False)
    extR_d = nc.declare_dram_parameter("extR", [KEXT, B, S], s2dt, isOutput=False)
    out_d = nc.declare_dram_parameter("out", [B, NT, 128, S], f32, isOutput=True)

    with tile.TileContext(nc) as tc:
        with (
            tc.tile_pool(name="const", bufs=1) as constp,
            tc.tile_pool(name="w1", bufs=4) as w1p,
            tc.tile_pool(name="ps", bufs=8, space="PSUM") as psp,
            tc.tile_pool(name="outp", bufs=2) as outp,
        ):
            predT_sb = constp.tile([128, DC, B * P], s1dt)

            # layout [e_part, ec, b, (l*64+p)] so stage-2 weight slices are
            # one contiguous 128-column run (walrus: weights AP must have a
            # single free dimension)
            tmpT_sb = constp.tile([128, ET, B, LSH * P], s2dt)
            spanT_sb = constp.tile([128, B, DC, S], s2dt)
            extL_sb = constp.tile([KEXT, B, NT, 128], s2dt)
            extR_sb = constp.tile([KEXT, B, S], s2dt)

            # ---- stage 1: tmpT[e, l, b, p] = sum_d W1[l,d,e] pred[b,p,d] ----
            # Iteration 0's operands arrive per-chunk, interleaved, so the
            # first matmul only waits for the c=0 pieces; spanT for batch
            # l-1 is prefetched during iteration l (resident before stage 2).
            NBP = B * P
            for l in range(LSH):
                w1t = w1p.tile([128, DC, D], s1dt, tag="w1")
                if l == 0:
                    for c in range(DC):
                        nc.sync.dma_start(
                            predT_sb[:, c, :], predT_d[:, c * NBP:(c + 1) * NBP]
                        )
                        nc.sync.dma_start(
                            w1t[:, c, :], w1s_d[0, :, c * D:(c + 1) * D]
                        )
                else:
                    nc.sync.dma_start(w1t[:], w1s_d[l])
                    nc.sync.dma_start(spanT_sb[:, l - 1, :, :], spanT_d[l - 1])
                if l == 1:
                    nc.sync.dma_start(extL_sb[:], extL_d[:])
                    nc.sync.dma_start(extR_sb[:], extR_d[:])
                for et in range(ET):
                    acc = psp.tile([128, 512], f32, tag="ps")
                    for c in range(DC):
                        nc.tensor.matmul(
                            acc[:],
                            w1t[:, c, et * 128:(et + 1) * 128],
                            predT_sb[:, c, :],
                            start=(c == 0),
                            stop=(c == DC - 1),
                        )
                    nc.vector.tensor_copy(
                        tmpT_sb[:, et, :, l * P:(l + 1) * P],
                        acc[:].rearrange("p (b q) -> p b q", b=B),
                    )

            nc.sync.dma_start(spanT_sb[:, B - 1, :, :], spanT_d[B - 1])

            # ---- stage 2: out[(l,p), s] = sum_e tmpT[e,(l,p)] spanT[e,s] + ext ----
            for b in range(B):
                ob = outp.tile([128, NT, S], f32, tag="ob")
                for t in range(NT):
                    acc2 = psp.tile([128, 512], f32, tag="ps")
                    nc.tensor.matmul(
                        acc2[:],
                        extL_sb[:, b, t, :],
                        extR_sb[:, b, :],
                        start=True,
                        stop=False,
                    )
                    for ec in range(ET):
                        nc.tensor.matmul(
                            acc2[:],
                            tmpT_sb[:, ec, b, t * 128:(t + 1) * 128],
                            spanT_sb[:, b, ec, :],
                            start=False,
                            stop=(ec == ET - 1),
                        )
                    nc.vector.tensor_copy(ob[:, t, :], acc2[:])
                    if b == B - 1:
                        # split the final batch's store so the drain tail
                        # only waits on the last quarter
                        nc.sync.dma_start(out_d[b, t], ob[:, t, :])
                if b < B - 1:
                    nc.sync.dma_start(out_d[b].rearrange("t m s -> m t s"), ob[:])

    nc.compile()
    return nc


def _host_prep(inputs):
    span_repr = np.asarray(inputs["span_repr"], dtype=np.float32)      # [B,S,D]
    predicates = np.asarray(inputs["predicates"])                       # [B,P]
    Wp = np.asarray(inputs["Wp"], dtype=np.float32)
    bp = np.asarray(inputs["bp"], dtype=np.float32)
    Wa = np.asarray(inputs["Wa"], dtype=np.float32)
    ba = np.asarray(inputs["ba"], dtype=np.float32)
    wp = np.asarray(inputs["wp"], dtype=np.float32)
    wa = np.asarray(inputs["wa"], dtype=np.float32)
    W2 = np.asarray(inputs["W2"], dtype=np.float32)
    bb = np.asarray(inputs["b"], dtype=np.float32)

    idx = predicates.astype(np.int64)[..., None]                        # [B,P,1]
    pred_repr = np.take_along_axis(span_repr, idx, axis=1)              # [B,P,D]

    pscore = np.maximum(pred_repr @ Wp.T + bp, 0.0) @ wp                # [B,P]
    ascore = np.maximum(span_repr @ Wa.T + ba, 0.0) @ wa                # [B,S]
    rowterm = pred_repr @ W2[:D] + bb[0] + pscore[..., None]            # [B,P,L]
    colterm = span_repr @ W2[D:] + ascore[..., None]                    # [B,S,L]

    spanT = span_repr.transpose(0, 2, 1)                                # [B,D,S]
    predT = pred_repr.reshape(B * P, D).T                               # [D,B*P]
    return spanT, predT, rowterm, colterm


def _pack(mat, np_dt):
    """[D, W] -> [128, DC*W] with partition p holding row d = c*128 + p."""
    d, w = mat.shape
    c = d // 128
    return np.ascontiguousarray(
        mat.reshape(c, 128, w).transpose(1, 0, 2).reshape(128, c * w)
    ).astype(np_dt, copy=False)


def _core_inputs(W1, spanT, predT, rowterm, colterm, np_s1, np_s2):
    predT_p = _pack(predT, np_s1)                                       # [128,DC*B*P]
    spanT_p = np.stack([_pack(spanT[b], np_s2) for b in range(B)])      # [B,128,DC*S]
    in_maps = []
    for c in range(NCORES):
        lsl = slice(c * LSH, (c + 1) * LSH)
        w1s = np.stack([_pack(W1[c * LSH + l], np_s1) for l in range(LSH)])

        eL = np.zeros([KEXT, B, NT, 128], np.float32)
        rt = rowterm[:, :, lsl]                                         # [B,P,LSH]
        for t in range(NT):
            for j in range(2):
                eL[2 * t + j, :, t, 64 * j:64 * (j + 1)] = 1.0
                eL[LSH, :, t, 64 * j:64 * (j + 1)] = rt[:, :, 2 * t + j]
        eR = np.empty([KEXT, B, S], np.float32)
        eR[:LSH] = colterm[:, :, lsl].transpose(2, 0, 1)
        eR[LSH] = 1.0

        in_maps.append({
            "spanT": spanT_p,
            "predT": predT_p,
            "w1s": w1s,
            "extL": eL.astype(np_s2, copy=False),
            "extR": eR.astype(np_s2, copy=False),
        })
    return in_maps


def _assemble(outs, labels):
    final = np.empty([B, P, S, L], np.float32)
    for c in range(NCORES):
        oc = np.asarray(outs[c]).reshape(B, LSH, P, S)
        final[:, :, :, c * LSH:(c + 1) * LSH] = oc.transpose(0, 2, 3, 1)
    final[..., L - 1] = 0.0
    return final.reshape(-1, L), np.asarray(labels).reshape(-1)


S1_DTYPE = "float16"
S2_DTYPE = "float16"

# test harness hooks: set TRACE=True before calling kernel() to profile;
# the BassKernelResults of the last run lands in LAST_RESULT.
TRACE = False
TRACE_CORES = None
LAST_RESULT = None


def _np_dtype(name):
    if name == "bfloat16":
        import ml_dtypes
        return np.dtype(ml_dtypes.bfloat16)
    if name == "float16":
        return np.dtype(np.float16)
    return np.dtype(np.float32)


def kernel(**inputs):
    from concourse.bass_utils import run_bass_kernel_spmd

    key = (S1_DTYPE, S2_DTYPE)
    if key not in _CACHE:
        _CACHE[key] = _build_module(S1_DTYPE, S2_DTYPE)
    nc = _CACHE[key]

    W1 = np.asarray(inputs["W1"], dtype=np.float32)
    spanT, predT, rowterm, colterm = _host_prep(inputs)
    in_maps = _core_inputs(
        W1, spanT, predT, rowterm, colterm,
        _np_dtype(S1_DTYPE), _np_dtype(S2_DTYPE),
    )
    res = run_bass_kernel_spmd(
        nc, in_maps, core_ids=list(range(NCORES)),
        trace=TRACE, trace_cores=TRACE_CORES,
    )
    globals()["LAST_RESULT"] = res
    outs = [r["out"] for r in res.results]
    return _assemble(outs, inputs["labels"])


# revision 37
# speedup vs baseline: 1.0196x; 1.0196x over previous
"""Trainium2 Bass kernel for the biaffine scorer (nn_Biaffine_overlap).

Reference computation (B=8, P=64, S=512, D=512, L=64):
    pred = gather(span_repr, predicates)                      # [B,P,D]
    first[b,p,s,l]  = sum_{d,e} pred[b,p,d] W1[l,d,e] span[b,s,e]
    combine         = first + rowterm[b,p,l] + colterm[b,s,l]
    out             = combine.reshape(-1, L); out[:, -1] = 0
where
    rowterm[b,p,l] = pred@W2[:D] + b + relu(pred@Wp.T+bp)@wp
    colterm[b,s,l] = span@W2[D:] + relu(span@Wa.T+ba)@wa

Sharding: tensor-parallel over the label dim L across 8 cores (8 labels
per core).  The W1 shard (8.4 MB) stays resident in SBUF; every core
processes all batches.  Per core, two dense matmul stages:
  stage 1: tmpT[e,(l,b,p)] = sum_d W1[l,d,e] predT[d,(b,p)]   (128 MMs)
  stage 2: out[(l,p),s]    = sum_e tmpT[e,(l,p)] spanT[e,s]   (128 MMs)
           + one K=9 "extension" matmul per output tile that folds in
           rowterm/colterm as a rank-9 update.
The small FFNN score terms (<6% of FLOPs) are computed on the host and
enter the device only through the rank-9 extension operands.
"""

import numpy as np

B, P, S, D, L = 8, 64, 512, 512, 64
NCORES = 8
LSH = L // NCORES      # labels per core
KEXT = LSH + 1         # extension contraction rows (8 colterm + 1 rowterm)
DC = D // 128          # contraction chunks of 128
ET = D // 128          # e-tiles of 128
NT = (LSH * P) // 128  # output row tiles per batch (4)

_CACHE: dict = {}


def _build_module(s1_dtype_name: str | None = None, s2_dtype_name: str | None = None):
    s1_dtype_name = s1_dtype_name or S1_DTYPE
    s2_dtype_name = s2_dtype_name or S2_DTYPE
    import concourse.bacc as bacc
    import concourse.mybir as mybir
    import concourse.tile as tile

    dt = mybir.dt
    f32 = dt.float32
    s1dt = getattr(dt, s1_dtype_name)   # stage-1 operand dtype (W1, predT)
    s2dt = getattr(dt, s2_dtype_name)   # stage-2 operand dtype (tmpT, spanT, ext)

    nc = bacc.Bacc(None, target_bir_lowering=False)

    # inputs are host-packed so every DMA lands in SBUF layout with long
    # contiguous per-partition runs: [128, DC, width] with partition p
    # holding row d = c*128 + p of the logical [D, width] matrix
    spanT_d = nc.declare_dram_parameter("spanT", [B, 128, DC * S], s2dt, isOutput=False)
    predT_d = nc.declare_dram_parameter("predT", [128, DC * B * P], s1dt, isOutput=False)
    w1s_d = nc.declare_dram_parameter("w1s", [LSH, 128, DC * D], s1dt, isOutput=False)
    # w2colT[p, et, l] = W2[D + et*128 + p, label l of this core's shard];
    # plane[b, m, (t, s)] = ascore[b, s] + rowterm[b, p(m), l(m, t)], host-
    # packed partition-major so the DMA is fully linear
    w2colT_d = nc.declare_dram_parameter("w2colT", [128, ET, LSH], f32, isOutput=False)
    plane_d = nc.declare_dram_parameter("plane", [B, 128, NT * S], s2dt, isOutput=False)
    out_d = nc.declare_dram_parameter("out", [B, NT, 128, S], f32, isOutput=True)

    with tile.TileContext(nc) as tc:
        with (
            tc.tile_pool(name="const", bufs=1) as constp,
            tc.tile_pool(name="w1", bufs=4) as w1p,
            tc.tile_pool(name="ps", bufs=8, space="PSUM") as psp,
            tc.tile_pool(name="outp", bufs=2) as outp,
        ):
            predT_sb = constp.tile([128, DC, B * P], s1dt)

            # layout [e_part, ec, b, (l*64+p)] so stage-2 weight slices are
            # one contiguous 128-column run (walrus: weights AP must have a
            # single free dimension)
            tmpT_sb = constp.tile([128, ET, B, LSH * P], s2dt)
            spanT_sb = constp.tile([128, B, DC, S], s2dt)
            w2colT_sb = constp.tile([128, ET, LSH], f32)
            plane_sb = constp.tile([128, B, NT, S], s2dt)

            # ---- stage 1: tmpT[e, l, b, p] = sum_d W1[l,d,e] pred[b,p,d] ----
            # Iteration 0's operands arrive per-chunk, interleaved, so the
            # first matmul only waits for the c=0 pieces; spanT for batch
            # l-1 is prefetched during iteration l (resident before stage 2).
            NBP = B * P
            for l in range(LSH):
                w1t = w1p.tile([128, DC, D], s1dt, tag="w1")
                if l == 0:
                    nc.sync.dma_start(w2colT_sb[:], w2colT_d[:])
                    for c in range(DC):
                        nc.sync.dma_start(
                            predT_sb[:, c, :], predT_d[:, c * NBP:(c + 1) * NBP]
                        )
                        nc.sync.dma_start(
                            w1t[:, c, :], w1s_d[0, :, c * D:(c + 1) * D]
                        )
                else:
                    nc.sync.dma_start(w1t[:], w1s_d[l])
                    nc.sync.dma_start(spanT_sb[:, l - 1, :, :], spanT_d[l - 1])
                    nc.sync.dma_start(
                        plane_sb[:, l - 1, :, :],
                        plane_d[l - 1].rearrange("m (t s) -> m t s", t=NT),
                    )
                for et in range(ET):
                    acc = psp.tile([128, 512], f32, tag="ps")
                    for c in range(DC):
                        nc.tensor.matmul(
                            acc[:],
                            w1t[:, c, et * 128:(et + 1) * 128],
                            predT_sb[:, c, :],
                            start=(c == 0),
                            stop=(c == DC - 1),
                        )
                    # PSUM -> SBUF copy fused with the W2-column fold:
                    # tmpT'[e,(l,b,p)] = acc + W2[D+e, l]
                    nc.vector.tensor_scalar_add(
                        tmpT_sb[:, et, :, l * P:(l + 1) * P],
                        acc[:].rearrange("p (b q) -> p b q", b=B),
                        w2colT_sb[:, et, l:l + 1],
                    )

            nc.sync.dma_start(spanT_sb[:, B - 1, :, :], spanT_d[B - 1])
            nc.sync.dma_start(
                plane_sb[:, B - 1, :, :],
                plane_d[B - 1].rearrange("m (t s) -> m t s", t=NT),
            )

            # ---- stage 2: out[(l,p), s] = sum_e tmpT'[e,(l,p)] spanT[e,s]
            #                               + plane[b,(l,p),s] ----
            for b in range(B):
                ob = outp.tile([128, NT, S], f32, tag="ob")
                for t in range(NT):
                    acc2 = psp.tile([128, 512], f32, tag="ps")
                    for ec in range(ET):
                        nc.tensor.matmul(
                            acc2[:],
                            tmpT_sb[:, ec, b, t * 128:(t + 1) * 128],
                            spanT_sb[:, b, ec, :],
                            start=(ec == 0),
                            stop=(ec == ET - 1),
                        )
                    # PSUM -> SBUF copy fused with the rowterm+ascore plane
                    nc.vector.tensor_tensor(
                        ob[:, t, :], acc2[:], plane_sb[:, b, t, :],
                        mybir.AluOpType.add,
                    )
                    if b == B - 1:
                        # split the final batch's store so the drain tail
                        # only waits on the last quarter
                        nc.sync.dma_start(out_d[b, t], ob[:, t, :])
                if b < B - 1:
                    nc.sync.dma_start(out_d[b].rearrange("t m s -> m t s"), ob[:])

    nc.compile()
    return nc


def _host_prep(inputs):
    span_repr = np.asarray(inputs["span_repr"], dtype=np.float32)      # [B,S,D]
    predicates = np.asarray(inputs["predicates"])                       # [B,P]
    Wp = np.asarray(inputs["Wp"], dtype=np.float32)
    bp = np.asarray(inputs["bp"], dtype=np.float32)
    Wa = np.asarray(inputs["Wa"], dtype=np.float32)
    ba = np.asarray(inputs["ba"], dtype=np.float32)
    wp = np.asarray(inputs["wp"], dtype=np.float32)
    wa = np.asarray(inputs["wa"], dtype=np.float32)
    W2 = np.asarray(inputs["W2"], dtype=np.float32)
    bb = np.asarray(inputs["b"], dtype=np.float32)

    idx = predicates.astype(np.int64)[..., None]                        # [B,P,1]
    pred_repr = np.take_along_axis(span_repr, idx, axis=1)              # [B,P,D]

    pscore = np.maximum(pred_repr @ Wp.T + bp, 0.0) @ wp                # [B,P]
    ascore = np.maximum(span_repr @ Wa.T + ba, 0.0) @ wa                # [B,S]
    rowterm = pred_repr @ W2[:D] + bb[0] + pscore[..., None]            # [B,P,L]

    spanT = span_repr.transpose(0, 2, 1)                                # [B,D,S]
    predT = pred_repr.reshape(B * P, D).T                               # [D,B*P]
    return spanT, predT, rowterm, ascore, W2


def _pack(mat, np_dt):
    """[D, W] -> [128, DC*W] with partition p holding row d = c*128 + p."""
    d, w = mat.shape
    c = d // 128
    return np.ascontiguousarray(
        mat.reshape(c, 128, w).transpose(1, 0, 2).reshape(128, c * w)
    ).astype(np_dt, copy=False)


def _core_inputs(W1, W2, spanT, predT, rowterm, ascore, np_s1, np_s2):
    predT_p = _pack(predT, np_s1)                                       # [128,DC*B*P]
    spanT_p = np.stack([_pack(spanT[b], np_s2) for b in range(B)])      # [B,128,DC*S]
    in_maps = []
    for c in range(NCORES):
        lsl = slice(c * LSH, (c + 1) * LSH)
        w1s = np.stack([_pack(W1[c * LSH + l], np_s1) for l in range(LSH)])

        # w2colT[p, et, l] = W2[D + et*128 + p, c*LSH + l]
        w2colT = np.ascontiguousarray(
            W2[D:, lsl].reshape(ET, 128, LSH).transpose(1, 0, 2)
        ).astype(np.float32)

        # plane[b, t, m, s] = ascore[b, s] + rowterm[b, p(m), 2t + (m>=64)]
        rt = rowterm[:, :, lsl]                                         # [B,P,LSH]
        # row vector per (b, t, m): arrange rowterm to [B, NT, 128]
        rv = rt.transpose(0, 2, 1).reshape(B, NT, 128)                  # [B,NT,(j,p)]
        plane = ascore[:, None, None, :] + rv[..., None]                # [B,NT,128,S]
        plane = np.ascontiguousarray(
            plane.transpose(0, 2, 1, 3).reshape(B, 128, NT * S)
        ).astype(np_s2, copy=False)

        in_maps.append({
            "spanT": spanT_p,
            "predT": predT_p,
            "w1s": w1s,
            "w2colT": w2colT,
            "plane": plane,
        })
    return in_maps


def _assemble(outs, labels):
    final = np.empty([B, P, S, L], np.float32)
    for c in range(NCORES):
        oc = np.asarray(outs[c]).reshape(B, LSH, P, S)
        final[:, :, :, c * LSH:(c + 1) * LSH] = oc.transpose(0, 2, 3, 1)
    final[..., L - 1] = 0.0
    return final.reshape(-1, L), np.asarray(labels).reshape(-1)


S1_DTYPE = "float16"
S2_DTYPE = "float16"

# test harness hooks: set TRACE=True before calling kernel() to profile;
# the BassKernelResults of the last run lands in LAST_RESULT.
TRACE = False
TRACE_CORES = None
LAST_RESULT = None


def _np_dtype(name):
    if name == "bfloat16":
        import ml_dtypes
        return np.dtype(ml_dtypes.bfloat16)
    if name == "float16":
        return np.dtype(np.float16)
    return np.dtype(np.float32)


def kernel(**inputs):
    from concourse.bass_utils import run_bass_kernel_spmd

    key = (S1_DTYPE, S2_DTYPE)
    if key not in _CACHE:
        _CACHE[key] = _build_module(S1_DTYPE, S2_DTYPE)
    nc = _CACHE[key]

    W1 = np.asarray(inputs["W1"], dtype=np.float32)
    spanT, predT, rowterm, ascore, W2 = _host_prep(inputs)
    in_maps = _core_inputs(
        W1, W2, spanT, predT, rowterm, ascore,
        _np_dtype(S1_DTYPE), _np_dtype(S2_DTYPE),
    )
    res = run_bass_kernel_spmd(
        nc, in_maps, core_ids=list(range(NCORES)),
        trace=TRACE, trace_cores=TRACE_CORES,
    )
    globals()["LAST_RESULT"] = res
    outs = [r["out"] for r in res.results]
    return _assemble(outs, inputs["labels"])
